# revision 1
# baseline (speedup 1.0000x reference)
"""DHASPI level-loss kernel for 8 Trainium2 NeuronCores.

Data-parallel over the fused B*C row axis: each of the 8 cores processes 64
rows of x_env and 64 rows of y_env (the x rows sit in SBUF partitions 0-63,
the y rows in partitions 64-127, so every DMA is a full 128-partition
transfer). Per row the kernel computes the gated LUFS loudness; the final
relu-diff scalar sum over the 512 rows is done on the host from the 8 tiny
[128, 1] per-core outputs.

Math notes:
- Frame energies (9600-sample windows, shift 2880) are built from 960-sample
  block sums: gcd(9600, 2880) = 960, frame f = blocks 3f..3f+9.
- Per 9600-sample chunk: the Scalar engine squares it (activation Square),
  the Vector engine then block-reduces [128, 10, 960] -> [128, 10]. The two
  engines pipeline across chunks; both stay under the HBM roofline, so the
  kernel is DMA-bound as intended for this memory-regime problem.
- All dB-domain gating comparisons are done in the energy domain via the
  monotone map el = -0.691 + 10*log10(z + eps):
    el > -70           <=>  z > 10**(-6.9309) - eps
    el > gamma_r       <=>  z > 0.1*(z_ave_a + eps) - eps
  so the only transcendental on device is one Ln per row at the end.

Written in raw Bass (explicit semaphores, double-buffered DMA + squares):
the Tile framework's kernel-tail drain emits multi-wait instructions this
walrus build rejects, and the fused accumulate features (tensor_tensor_reduce,
accum_out) are dropped by its codegen — so plain BIR ops with manual sync.
"""

import math

import numpy as np

import concourse.bass as bass
from concourse import mybir
from concourse.bass_utils import run_bass_kernel_spmd

# Problem constants (hardcoded from the spec; kernel.py must be self-contained)
B, C, T = 16, 32, 192000
N_CORES = 8
ROWS = B * C  # 512
RPC = ROWS // N_CORES  # 64 rows per core per tensor

FRAME = 9600
SHIFT = 2880
BLK = 960  # gcd(FRAME, SHIFT)
NBLK = T // BLK  # 200 block sums per row
NFRM = (T - FRAME) // SHIFT + 1  # 64 frames per row
CHUNK = 9600  # chunk size (4.9 MB per 128-row transfer)
# The last main chunk is split into small tail chunks so the final
# square+reduce after the last DMA is short (~4 us instead of ~19 us).
CHUNKS = [CHUNK] * (T // CHUNK - 1) + [1920] * (CHUNK // 1920)
NBUF = 2  # input chunk buffers (and squared-chunk buffers)

EPS = 1e-8
ALPHA = 1e-4
GAMMA_A = -70.0
# z-domain threshold equivalent to el > GAMMA_A
TA = float(10.0 ** ((GAMMA_A + 0.691) / 10.0) - EPS)
# relative threshold: z > 0.1*(z_ave_a + EPS) - EPS = 0.1*z_ave_a + TR_OFF
TR_OFF = float(0.1 * EPS - EPS)
LN10_INV10 = float(10.0 / math.log(10.0))
INV_FRAME = float(1.0 / FRAME)

F32 = mybir.dt.float32


def _overlapped_frames_view(bs_ap):
    """[128, NFRM, 10] view of the block-sum tile: frame f = blocks 3f..3f+9."""
    base = bs_ap[:, 0:1]
    return type(base)(
        tensor=base.tensor,
        offset=base.offset,
        ap=[list(base.ap[0]), [3, NFRM], [1, FRAME // BLK]],
    )


def _build_program(debug_stop: str | None = None) -> bass.Bass:
    """debug_stop: None=full kernel, 'loop'=skip epilogue (dev-only knob)."""
    nc = bass.Bass("TRN2", target_bir_lowering=False, debug=False)
    AF = mybir.ActivationFunctionType
    ALU = mybir.AluOpType
    AX = mybir.AxisListType

    xy = nc.dram_tensor("xy", [128, T], F32, kind="ExternalInput").ap()
    out = nc.dram_tensor("lufs", [128, 1], F32, kind="ExternalOutput").ap()

    # SBUF working set, per partition: 2*37.5KB input + 2*37.5KB squared
    # + ~3KB small tiles = ~153KB of the 192KB budget.
    xt = [nc.alloc_sbuf_tensor(f"xt{i}", [128, CHUNK], F32).ap() for i in range(NBUF)]
    sq = [nc.alloc_sbuf_tensor(f"sq{i}", [128, CHUNK], F32).ap() for i in range(NBUF)]
    bs = nc.alloc_sbuf_tensor("bs", [128, NBLK], F32).ap()
    zsum = nc.alloc_sbuf_tensor("zsum", [128, NFRM], F32).ap()
    z = nc.alloc_sbuf_tensor("z", [128, NFRM], F32).ap()
    ga = nc.alloc_sbuf_tensor("ga", [128, NFRM], F32).ap()
    ma = nc.alloc_sbuf_tensor("ma", [128, NFRM], F32).ap()
    gar = nc.alloc_sbuf_tensor("gar", [128, NFRM], F32).ap()
    junk = nc.alloc_sbuf_tensor("junk", [128, NFRM], F32).ap()
    sc = nc.alloc_sbuf_tensor("sc", [128, 12], F32).ap()  # per-row scalars
    eps_t = nc.alloc_sbuf_tensor("eps_t", [128, 1], F32).ap()

    numa = sc[:, 0:1]
    dena = sc[:, 1:2]
    rca = sc[:, 2:3]
    zavea = sc[:, 3:4]
    thr = sc[:, 4:5]
    denar = sc[:, 5:6]
    numar = sc[:, 6:7]
    rcar = sc[:, 7:8]
    zavear = sc[:, 8:9]
    lnz = sc[:, 9:10]
    lufs_t = sc[:, 10:11]

    with (
        nc.Block() as block,
        nc.semaphore("dma_sem0") as dma_sem0,
        nc.semaphore("dma_sem1") as dma_sem1,
        nc.semaphore("dma_out_sem") as dma_out_sem,
        nc.semaphore("act_sem") as act_sem,
        nc.semaphore("dve_sem") as dve_sem,
    ):
        # One DMA-completion sem per buffer slot: a shared cumulative sem is
        # unsafe with >1 DMA in flight (a later DMA's per-engine increments
        # can reach the threshold while an earlier DMA is still draining).
        dma_sems = [dma_sem0, dma_sem1]

        n_chunks = len(CHUNKS)
        offs = [sum(CHUNKS[:i]) for i in range(n_chunks)]

        @block.sync
        def _(sync):
            for c, (off, size) in enumerate(zip(offs, CHUNKS)):
                if c >= NBUF:
                    # input slot c%NBUF is free once ACT has squared chunk c-NBUF
                    sync.wait_ge(act_sem, c - NBUF + 1)
                sync.dma_start(
                    out=xt[c % NBUF][:, 0:size], in_=xy[:, off : off + size]
                ).then_inc(dma_sems[c % NBUF], 16)
            # final output DMA after the epilogue chain finishes
            sync.wait_ge(dve_sem, n_chunks + 2)
            sync.dma_start(out=out, in_=lufs_t).then_inc(dma_out_sem, 16)
            sync.wait_ge(dma_out_sem, 16)

        @block.scalar
        def _(scalar):
            for c, size in enumerate(CHUNKS):
                scalar.wait_ge(dma_sems[c % NBUF], (c // NBUF + 1) * 16)
                if c >= NBUF:
                    # sq slot c%NBUF is free once DVE has reduced chunk c-NBUF
                    scalar.wait_ge(dve_sem, c - NBUF + 1)
                scalar.activation(
                    sq[c % NBUF][:, 0:size], xt[c % NBUF][:, 0:size], AF.Square
                )
                # flush the pipe before signalling so DVE reads landed data
                scalar.drain().then_inc(act_sem, 1)
            # epilogue: ln(z_ave_ar + EPS) once DVE has produced z_ave_ar
            scalar.wait_ge(dve_sem, n_chunks + 1)
            scalar.activation(lnz, zavear, AF.Ln, bias=eps_t)
            scalar.drain().then_inc(act_sem, 1)

        @block.vector
        def _(vector):
            ALU_ = ALU
            vector.memset(eps_t, EPS)
            for c, (off, size) in enumerate(zip(offs, CHUNKS)):
                vector.wait_ge(act_sem, c + 1)
                sqv = sq[c % NBUF][:, 0:size].rearrange("p (n b) -> p n b", b=BLK)
                vector.reduce_sum(
                    bs[:, off // BLK : (off + size) // BLK], sqv, axis=AX.X
                ).then_inc(dve_sem, 1)

            # ---- epilogue (all [128, NFRM] or [128, 1] ops) ----
            # Raw-bass DVE instructions race on same-engine RAW (no implicit
            # pipeline flush between instructions on this HW), so drain()
            # between every dependent pair. ~12 drains ≈ a few µs, off the
            # critical path.
            if debug_stop == "loop":
                vector.memset(zavear, 1.0)
                vector.drain().then_inc(dve_sem, 1)
                vector.wait_ge(act_sem, n_chunks + 1)
                vector.tensor_scalar_mul(lufs_t, lnz, 1.0)
                vector.drain().then_inc(dve_sem, 1)
                return
            # z[f] = (sum of blocks 3f..3f+9) / FRAME
            vector.drain()
            vector.reduce_sum(zsum[:, :], _overlapped_frames_view(bs), axis=AX.X)
            vector.drain()
            vector.tensor_scalar_mul(z[:, :], zsum[:, :], INV_FRAME)
            vector.drain()
            # absolute gating: ma = (z > TA), ga = ma * z
            vector.scalar_tensor_tensor(
                out=ga[:, :], in0=z[:, :], scalar=TA, in1=z[:, :],
                op0=ALU_.is_gt, op1=ALU_.mult,
            )
            vector.tensor_scalar(ma[:, :], z[:, :], TA, None, op0=ALU_.is_gt)
            vector.drain()
            vector.reduce_sum(numa, ga[:, :], axis=AX.X)
            vector.reduce_sum(dena, ma[:, :], axis=AX.X)
            vector.drain()
            # z_ave_a = numa / (dena + EPS); relative threshold
            vector.tensor_scalar_add(dena, dena, EPS)
            vector.drain()
            vector.reciprocal(rca, dena)
            vector.drain()
            vector.tensor_tensor(zavea, numa, rca, op=ALU_.mult)
            vector.drain()
            vector.tensor_scalar(thr, zavea, 0.1, TR_OFF, op0=ALU_.mult, op1=ALU_.add)
            vector.drain()
            # relative gating: gar = (z > thr) * ma; numar = sum(z * gar)
            vector.scalar_tensor_tensor(
                out=gar[:, :], in0=z[:, :], scalar=thr, in1=ma[:, :],
                op0=ALU_.is_gt, op1=ALU_.mult,
            )
            vector.drain()
            # z*gar = (z > thr) * ga, so reuse ga instead of a fresh multiply
            vector.scalar_tensor_tensor(
                out=junk[:, :], in0=z[:, :], scalar=thr, in1=ga[:, :],
                op0=ALU_.is_gt, op1=ALU_.mult,
            )
            vector.reduce_sum(denar, gar[:, :], axis=AX.X)
            vector.drain()
            vector.reduce_sum(numar, junk[:, :], axis=AX.X)
            vector.drain()
            # z_ave_ar = numar / (denar + EPS)
            vector.tensor_scalar_add(denar, denar, EPS)
            vector.drain()
            vector.reciprocal(rcar, denar)
            vector.drain()
            vector.tensor_tensor(zavear, numar, rcar, op=ALU_.mult)
            vector.drain().then_inc(dve_sem, 1)
            # lufs = -0.691 + (10/ln10) * ln(z_ave_ar + EPS); ln from ACT
            vector.wait_ge(act_sem, n_chunks + 1)
            vector.tensor_scalar(
                lufs_t, lnz, LN10_INV10, -0.691, op0=ALU_.mult, op1=ALU_.add
            )
            vector.drain().then_inc(dve_sem, 1)

    return nc


def make_in_maps(x_env: np.ndarray, y_env: np.ndarray) -> list[dict[str, np.ndarray]]:
    x = np.asarray(x_env, dtype=np.float32).reshape(ROWS, T)
    y = np.asarray(y_env, dtype=np.float32).reshape(ROWS, T)
    in_maps = []
    for i in range(N_CORES):
        shard = np.concatenate(
            [x[i * RPC : (i + 1) * RPC], y[i * RPC : (i + 1) * RPC]], axis=0
        )
        in_maps.append({"xy": np.ascontiguousarray(shard)})
    return in_maps


def finish(per_core_lufs: list[np.ndarray]) -> np.ndarray:
    total = 0.0
    for lf in per_core_lufs:
        lf = np.asarray(lf).reshape(128).astype(np.float64)
        total += np.maximum(lf[RPC:] - lf[:RPC], 0.0).sum()
    return np.array(ALPHA * total, dtype=np.float32)


def kernel(x_env: np.ndarray, y_env: np.ndarray) -> np.ndarray:
    nc = _build_program()
    in_maps = make_in_maps(x_env, y_env)
    res = run_bass_kernel_spmd(nc, in_maps, core_ids=list(range(N_CORES)))
    return finish([res.results[i]["lufs"] for i in range(N_CORES)])



# revision 6
# speedup vs baseline: 1.6901x; 1.6901x over previous
"""DHASPI level-loss kernel for 8 Trainium2 NeuronCores.

Data-parallel over the fused B*C row axis: each core processes 64 rows of
x_env (SBUF partitions 0-63) and 64 rows of y_env (partitions 64-127). Per
row the kernel computes the gated LUFS loudness; the final relu-diff scalar
sum over the 512 rows is done on the host from the 8 tiny [128, 1] outputs.

Math notes:
- Frame energies (9600-sample windows, shift 2880) are built from 960-sample
  block sums: gcd(9600, 2880) = 960, frame f = blocks 3f..3f+9. Only blocks
  0..198 are covered by any frame, so the last 960 samples of every row are
  never loaded (191040 of 192000 samples).
- All dB-domain gating comparisons are done in the energy domain via the
  monotone map el = -0.691 + 10*log10(z + eps), so the only transcendental
  on device is one Ln per row at the end.

Engine-balanced dataflow (per 960-sample block, all four queues ~equal):
- Loads are split between the SP queue (f32) and the GpSimd/Pool queue
  (SWDGE f32->bf16 cast during DMA, half the SBUF write traffic).
- Squares are split between ACT (activation Square, bf16 out) and DVE
  (tensor_tensor mult in bf16, 2x perf mode).
- The 960->60 per-block reduction runs as an in-place binary fold tree on
  the squared tile (bf16 tensor_tensor adds at 2x; lo += hi within each
  block), with the first fold optionally done by a Pool SBUF->SBUF DMA
  with accum_op=add. A final 1x tensor_reduce collapses 60 -> 1.
- bf16 block sums carry ~7e-4 relative error -> ~0.003 dB on LUFS, far
  inside the 2e-2 gate.

Raw Bass (explicit semaphores): same-queue DMA completions are FIFO, so one
cumulative semaphore per DMA stream is safe; cross-engine RAW uses drain()
before then_inc like the original kernel.
"""

import math

import numpy as np

import concourse.bass as bass
from concourse import mybir
from concourse.bass_utils import run_bass_kernel_spmd

# Problem constants (hardcoded from the spec; kernel.py must be self-contained)
B, C, T = 16, 32, 192000
N_CORES = 8
ROWS = B * C  # 512
RPC = ROWS // N_CORES  # 64 rows per core per tensor

FRAME = 9600
SHIFT = 2880
BLK = 960
NBLK_USED = 199  # blocks 0..198 feed frames; block 199 is dead
NFRM = (T - FRAME) // SHIFT + 1  # 64

EPS = 1e-8
ALPHA = 1e-4
GAMMA_A = -70.0
TA = float(10.0 ** ((GAMMA_A + 0.691) / 10.0) - EPS)  # z-domain abs threshold
TR_OFF = float(0.1 * EPS - EPS)
LN10_INV10 = float(10.0 / math.log(10.0))
INV_FRAME = float(1.0 / FRAME)

F32 = mybir.dt.float32
BF16 = mybir.dt.bfloat16

# ---- chunk schedule -------------------------------------------------------
# sizes in blocks; 23 chunks totalling 199 blocks. Small chunks at the start
# (fast pipeline fill) and end (short post-last-DMA critical path).
CHUNK_BLOCKS = [5, 5] + [10] * 18 + [5, 3, 1]
# which chunks the SP queue loads as f32 (the rest are Pool bf16-cast loads)
SP_LOAD = {4, 6, 8, 10, 12, 14, 16, 18}
# which chunks DVE squares (must be Pool-cast loads); the rest ACT squares
DVE_SQ = {2, 7, 13, 19, 20, 21, 22}
# which chunks get fold-1 from a Pool SBUF->SBUF accum DMA (ACT-squared only)
POOL_FOLD = {3, 5, 6, 9, 10, 11, 12, 14, 15, 17}

N_F32 = 2  # f32 input tile slots (SP loads)
N_B16 = 3  # bf16 input tile slots (Pool cast loads)
N_SQ = 3  # squared-tile slots (shared, round-robin in global chunk order)
MAXW = 10 * BLK  # widest chunk


def _sched():
    """Static schedule bookkeeping shared by all engine programs."""
    n = len(CHUNK_BLOCKS)
    off = [sum(CHUNK_BLOCKS[:i]) for i in range(n)]
    sp_list = [c for c in range(n) if c in SP_LOAD]
    pool_list = [c for c in range(n) if c not in SP_LOAD]
    act_sq = [c for c in range(n) if c not in DVE_SQ]
    pool_fold = [c for c in range(n) if c in POOL_FOLD]
    return n, off, sp_list, pool_list, act_sq, pool_fold


def _frames_view(bs_ap):
    """[128, NFRM, 10] view of the block-sum tile: frame f = blocks 3f..3f+9."""
    base = bs_ap[:, 0:1]
    return type(base)(
        tensor=base.tensor,
        offset=base.offset,
        ap=[list(base.ap[0]), [3, NFRM], [1, FRAME // BLK]],
    )


def _build_program() -> bass.Bass:
    nc = bass.Bass("TRN2", target_bir_lowering=False, debug=False)
    AF = mybir.ActivationFunctionType
    ALU = mybir.AluOpType
    AX = mybir.AxisListType

    n, off, sp_list, pool_list, act_sq, pool_fold = _sched()
    # ordinal of each chunk within the list that owns its square / reduce
    act_ord = {c: i for i, c in enumerate(act_sq)}
    pool_fold_ord = {c: i for i, c in enumerate(pool_fold)}
    sp_ord = {c: i for i, c in enumerate(sp_list)}
    pool_ord = {c: i for i, c in enumerate(pool_list)}

    xy = nc.dram_tensor("xy", [128, T], F32, kind="ExternalInput").ap()
    out = nc.dram_tensor("lufs", [128, 1], F32, kind="ExternalOutput").ap()

    xt32 = [
        nc.alloc_sbuf_tensor(f"xt32_{i}", [128, MAXW], F32).ap() for i in range(N_F32)
    ]
    xt16 = [
        nc.alloc_sbuf_tensor(f"xt16_{i}", [128, MAXW], BF16).ap() for i in range(N_B16)
    ]
    sq = [
        nc.alloc_sbuf_tensor(f"sq_{i}", [128, MAXW], BF16).ap() for i in range(N_SQ)
    ]
    bs = nc.alloc_sbuf_tensor("bs", [128, NBLK_USED], F32).ap()
    zsum = nc.alloc_sbuf_tensor("zsum", [128, NFRM], F32).ap()
    z = nc.alloc_sbuf_tensor("z", [128, NFRM], F32).ap()
    ga = nc.alloc_sbuf_tensor("ga", [128, NFRM], F32).ap()
    ma = nc.alloc_sbuf_tensor("ma", [128, NFRM], F32).ap()
    gar = nc.alloc_sbuf_tensor("gar", [128, NFRM], F32).ap()
    junk = nc.alloc_sbuf_tensor("junk", [128, NFRM], F32).ap()
    sc = nc.alloc_sbuf_tensor("sc", [128, 12], F32).ap()
    eps_t = nc.alloc_sbuf_tensor("eps_t", [128, 1], F32).ap()

    numa = sc[:, 0:1]
    dena = sc[:, 1:2]
    rca = sc[:, 2:3]
    zavea = sc[:, 3:4]
    thr = sc[:, 4:5]
    denar = sc[:, 5:6]
    numar = sc[:, 6:7]
    rcar = sc[:, 7:8]
    zavear = sc[:, 8:9]
    lnz = sc[:, 9:10]
    lufs_t = sc[:, 10:11]

    def blkv(ap, w, b=BLK):
        """[128, w//b, b] block view of the first w columns of a tile."""
        return ap[:, 0:w].rearrange("p (n b) -> p n b", b=b)

    with (
        nc.Block() as block,
        nc.semaphore("s_f32_0") as s_f32_0,
        nc.semaphore("s_f32_1") as s_f32_1,
        nc.semaphore("s_b16_0") as s_b16_0,
        nc.semaphore("s_b16_1") as s_b16_1,
        nc.semaphore("s_b16_2") as s_b16_2,
        nc.semaphore("s_sqA") as s_sqA,
        nc.semaphore("s_sqD") as s_sqD,
        nc.semaphore("s_pf0") as s_pf0,
        nc.semaphore("s_pf1") as s_pf1,
        nc.semaphore("s_red") as s_red,
        nc.semaphore("s_zav") as s_zav,
        nc.semaphore("s_ln") as s_ln,
        nc.semaphore("s_lufs") as s_lufs,
        nc.semaphore("s_out") as s_out,
    ):
        s_f32 = [s_f32_0, s_f32_1]
        s_b16 = [s_b16_0, s_b16_1, s_b16_2]
        # Rotating completion sems for the Pool fold DMAs. A single cumulative
        # DMA sem is unsafe with >1 DMA in flight (a later DMA's per-engine
        # increments can reach the threshold while an earlier one is still
        # draining), so fold k uses sem k%2 and its issue is gated on the
        # consumption (chunk reduce) of fold k-2 — at most 2 in flight.
        s_pf = [s_pf0, s_pf1]

        def sq_done_wait(eng, c):
            """Make `eng` wait until chunk c's square has landed."""
            if c in DVE_SQ:
                eng.wait_ge(s_sqD, sum(1 for d in DVE_SQ if d <= c))
            else:
                eng.wait_ge(s_sqA, act_ord[c] + 1)

        # ---- SP: f32 loads + final output DMA -----------------------------
        @block.sync
        def _(sy):
            for i, c in enumerate(sp_list):
                w = CHUNK_BLOCKS[c] * BLK
                slot = i % N_F32
                if i >= N_F32:
                    # slot free once ACT squared its previous occupant
                    prev = sp_list[i - N_F32]
                    sy.wait_ge(s_sqA, act_ord[prev] + 1)
                sy.dma_start(
                    out=xt32[slot][:, 0:w], in_=xy[:, off[c] * BLK : off[c] * BLK + w]
                ).then_inc(s_f32[slot], 16)
            sy.wait_ge(s_lufs, 1)
            sy.dma_start(out=out, in_=lufs_t).then_inc(s_out, 16)
            sy.wait_ge(s_out, 16)

        # ---- Pool: bf16 cast loads + fold-1 accum DMAs --------------------
        @block.gpsimd
        def _(g):
            emitted = set()

            def maybe_fold(c):
                if c in POOL_FOLD and c not in emitted:
                    emitted.add(c)
                    k = pool_fold_ord[c]
                    w = CHUNK_BLOCKS[c] * BLK
                    t = sq[c % N_SQ]
                    if k >= 2:
                        g.wait_ge(s_red, pool_fold[k - 2] + 1)
                    g.wait_ge(s_sqA, act_ord[c] + 1)
                    g.dma_start(
                        out=blkv(t, w)[:, :, 0 : BLK // 2],
                        in_=blkv(t, w)[:, :, BLK // 2 : BLK],
                        accum_op=mybir.AluOpType.add,
                    ).then_inc(s_pf[k % 2], 16)

            for i, c in enumerate(pool_list):
                w = CHUNK_BLOCKS[c] * BLK
                slot = i % N_B16
                if i >= N_B16:
                    prev = pool_list[i - N_B16]
                    sq_done_wait(g, prev)
                g.dma_start(
                    out=xt16[slot][:, 0:w], in_=xy[:, off[c] * BLK : off[c] * BLK + w]
                ).then_inc(s_b16[slot], 16)
                # fold DMAs trail the load stream by ~2 chunks so their
                # square-done waits are already satisfied
                for f in pool_fold:
                    if f <= c - 2:
                        maybe_fold(f)
            for f in pool_fold:
                maybe_fold(f)

        # ---- ACT: squares (any input dtype -> bf16) -----------------------
        @block.scalar
        def _(s):
            for i, c in enumerate(act_sq):
                w = CHUNK_BLOCKS[c] * BLK
                if c in SP_LOAD:
                    si = sp_ord[c]
                    s.wait_ge(s_f32[si % N_F32], (si // N_F32 + 1) * 16)
                    src = xt32[si % N_F32]
                else:
                    pi = pool_ord[c]
                    s.wait_ge(s_b16[pi % N_B16], (pi // N_B16 + 1) * 16)
                    src = xt16[pi % N_B16]
                if c >= N_SQ:
                    # sq slot free once its previous occupant was reduced
                    s.wait_ge(s_red, c - N_SQ + 1)
                s.activation(sq[c % N_SQ][:, 0:w], src[:, 0:w], AF.Square)
                s.drain().then_inc(s_sqA, 1)
            # epilogue: ln(z_ave_ar + EPS)
            s.wait_ge(s_zav, 1)
            s.activation(lnz, zavear, AF.Ln, bias=eps_t)
            s.drain().then_inc(s_ln, 1)

        # ---- DVE: squares (bf16), fold tree, reduces, gating epilogue -----
        @block.vector
        def _(v):
            v.memset(eps_t, EPS)
            for c in range(n):
                nb = CHUNK_BLOCKS[c]
                w = nb * BLK
                t = sq[c % N_SQ]
                if c in DVE_SQ:
                    pi = pool_ord[c]
                    v.wait_ge(s_b16[pi % N_B16], (pi // N_B16 + 1) * 16)
                    src = xt16[pi % N_B16]
                    # sq-slot reuse is implied: this queue ran the previous
                    # occupant's reduce earlier in program order
                    v.tensor_tensor(t[:, 0:w], src[:, 0:w], src[:, 0:w], op=ALU.mult)
                    v.drain().then_inc(s_sqD, 1)
                    wcur = BLK
                elif c in POOL_FOLD:
                    k = pool_fold_ord[c]
                    v.wait_ge(s_pf[k % 2], (k // 2 + 1) * 16)
                    wcur = BLK // 2
                else:
                    v.wait_ge(s_sqA, act_ord[c] + 1)
                    wcur = BLK
                # in-place fold tree: lo half += hi half within each block
                while wcur > 60:
                    h = wcur // 2
                    bv = blkv(t, w)
                    v.tensor_tensor(
                        bv[:, :, 0:h], bv[:, :, 0:h], bv[:, :, h:wcur], op=ALU.add
                    )
                    v.drain()
                    wcur = h
                v.reduce_sum(
                    bs[:, off[c] : off[c] + nb], blkv(t, w)[:, :, 0:60], axis=AX.X
                )
                v.drain().then_inc(s_red, 1)

            # ---- gating epilogue (all [128, NFRM] or [128, 1] ops) --------
            v.reduce_sum(zsum[:, :], _frames_view(bs), axis=AX.X)
            v.drain()
            v.tensor_scalar_mul(z[:, :], zsum[:, :], INV_FRAME)
            v.drain()
            v.scalar_tensor_tensor(
                out=ga[:, :], in0=z[:, :], scalar=TA, in1=z[:, :],
                op0=ALU.is_gt, op1=ALU.mult,
            )
            v.tensor_scalar(ma[:, :], z[:, :], TA, None, op0=ALU.is_gt)
            v.drain()
            v.reduce_sum(numa, ga[:, :], axis=AX.X)
            v.reduce_sum(dena, ma[:, :], axis=AX.X)
            v.drain()
            v.tensor_scalar_add(dena, dena, EPS)
            v.drain()
            v.reciprocal(rca, dena)
            v.drain()
            v.tensor_tensor(zavea, numa, rca, op=ALU.mult)
            v.drain()
            v.tensor_scalar(thr, zavea, 0.1, TR_OFF, op0=ALU.mult, op1=ALU.add)
            v.drain()
            v.scalar_tensor_tensor(
                out=gar[:, :], in0=z[:, :], scalar=thr, in1=ma[:, :],
                op0=ALU.is_gt, op1=ALU.mult,
            )
            v.drain()
            # z*gar = (z > thr) * ga, so reuse ga instead of a fresh multiply
            v.scalar_tensor_tensor(
                out=junk[:, :], in0=z[:, :], scalar=thr, in1=ga[:, :],
                op0=ALU.is_gt, op1=ALU.mult,
            )
            v.reduce_sum(denar, gar[:, :], axis=AX.X)
            v.drain()
            v.reduce_sum(numar, junk[:, :], axis=AX.X)
            v.drain()
            v.tensor_scalar_add(denar, denar, EPS)
            v.drain()
            v.reciprocal(rcar, denar)
            v.drain()
            v.tensor_tensor(zavear, numar, rcar, op=ALU.mult)
            v.drain().then_inc(s_zav, 1)
            v.wait_ge(s_ln, 1)
            v.tensor_scalar(
                lufs_t, lnz, LN10_INV10, -0.691, op0=ALU.mult, op1=ALU.add
            )
            v.drain().then_inc(s_lufs, 1)

    return nc


def make_in_maps(x_env: np.ndarray, y_env: np.ndarray) -> list[dict[str, np.ndarray]]:
    x = np.asarray(x_env, dtype=np.float32).reshape(ROWS, T)
    y = np.asarray(y_env, dtype=np.float32).reshape(ROWS, T)
    in_maps = []
    for i in range(N_CORES):
        shard = np.concatenate(
            [x[i * RPC : (i + 1) * RPC], y[i * RPC : (i + 1) * RPC]], axis=0
        )
        in_maps.append({"xy": np.ascontiguousarray(shard)})
    return in_maps


def finish(per_core_lufs: list[np.ndarray]) -> np.ndarray:
    total = 0.0
    for lf in per_core_lufs:
        lf = np.asarray(lf).reshape(128).astype(np.float64)
        total += np.maximum(lf[RPC:] - lf[:RPC], 0.0).sum()
    return np.array(ALPHA * total, dtype=np.float32)


def kernel(x_env: np.ndarray, y_env: np.ndarray) -> np.ndarray:
    nc = _build_program()
    in_maps = make_in_maps(x_env, y_env)
    res = run_bass_kernel_spmd(nc, in_maps, core_ids=list(range(N_CORES)))
    return finish([res.results[i]["lufs"] for i in range(N_CORES)])


# revision 9
# speedup vs baseline: 1.7035x; 1.0079x over previous
"""DHASPI level-loss kernel for 8 Trainium2 NeuronCores.

Data-parallel over the fused B*C row axis: each core processes 64 rows of
x_env (SBUF partitions 0-63) and 64 rows of y_env (partitions 64-127). Per
row the kernel computes the gated LUFS loudness; the final relu-diff scalar
sum over the 512 rows is done on the host from the 8 tiny [128, 1] outputs.

Math notes:
- Frame energies (9600-sample windows, shift 2880) are built from 960-sample
  block sums: gcd(9600, 2880) = 960, frame f = blocks 3f..3f+9. Only blocks
  0..198 are covered by any frame, so the last 960 samples of every row are
  never loaded (191040 of 192000 samples).
- All dB-domain gating comparisons are done in the energy domain via the
  monotone map el = -0.691 + 10*log10(z + eps), so the only transcendental
  on device is one Ln per row at the end.
- bf16 squares/folds put ~2e-4 relative noise on block sums (~0.001 dB on
  LUFS); the gating margins on this problem are >9 dB, far from any flip.

Engine-balanced dataflow (all four queues ~equal):
- Loads split between the SP queue (f32) and the GpSimd/Pool queue (SWDGE
  f32->bf16 cast during DMA, half the SBUF write traffic).
- Squares split between ACT (activation Square) and DVE (tensor_tensor mult
  in bf16, 2x perf mode). Squares are IN-PLACE: the input tile is squared
  into itself (for f32 inputs, into the tile's low half via a bf16 bitcast
  view - the bf16 write offset 2i trails the f32 read offset 4i, so the
  stream never clobbers unread data).
- The 960->60 per-block reduction is an in-place binary fold tree (bf16
  tensor_tensor adds at 2x; lo += hi within each block); fold-1 of some
  chunks is offloaded to Pool SBUF->SBUF DMAs with accum_op=add. A final
  1x tensor_reduce collapses 60 -> 1 per block into f32 block sums.

Raw Bass (explicit semaphores). Same-queue DMA completions are only safe to
track on one cumulative semaphore when never >1 in flight, so the Pool fold
DMAs rotate over two sems with issue gated on consumption of fold k-2.
"""

import math

import numpy as np

import concourse.bass as bass
from concourse import mybir
from concourse.bass_utils import run_bass_kernel_spmd

# Problem constants (hardcoded from the spec; kernel.py must be self-contained)
B, C, T = 16, 32, 192000
N_CORES = 8
ROWS = B * C  # 512
RPC = ROWS // N_CORES  # 64 rows per core per tensor

FRAME = 9600
SHIFT = 2880
BLK = 960
NBLK_USED = 199  # blocks 0..198 feed frames; block 199 is dead
NFRM = (T - FRAME) // SHIFT + 1  # 64

EPS = 1e-8
ALPHA = 1e-4
GAMMA_A = -70.0
TA = float(10.0 ** ((GAMMA_A + 0.691) / 10.0) - EPS)  # z-domain abs threshold
TR_OFF = float(0.1 * EPS - EPS)
LN10_INV10 = float(10.0 / math.log(10.0))
INV_FRAME = float(1.0 / FRAME)

F32 = mybir.dt.float32
BF16 = mybir.dt.bfloat16

# ---- chunk schedule -------------------------------------------------------
# sizes in blocks; 23 chunks totalling 199 blocks. Small chunks at the start
# (fast pipeline fill from cheap Pool cast loads) and at the end (short
# post-last-DMA critical path: tiny DVE square + folds, then the epilogue).
CHUNK_BLOCKS = [5, 3, 1] + [10] * 18 + [7, 3]
# which chunks the SP queue loads as f32 (the rest are Pool bf16-cast loads)
SP_LOAD = {4, 6, 8, 10, 12, 14, 16, 18}
# which chunks DVE squares (must be Pool-cast loads); the rest ACT squares
DVE_SQ = {1, 2, 5, 9, 13, 17, 21, 22}
# which chunks get fold-1 from a Pool SBUF->SBUF accum DMA (ACT-squared,
# kept away from the first and last chunks)
POOL_FOLD = {6, 7, 8, 10, 11, 12, 14, 15, 16}

N_F32 = 2  # f32 input tile slots (SP loads)
N_B16 = 5  # bf16 input tile slots (Pool cast loads)
MAXW = 10 * BLK  # widest chunk


def _sched():
    n = len(CHUNK_BLOCKS)
    off = [sum(CHUNK_BLOCKS[:i]) for i in range(n)]
    sp_list = [c for c in range(n) if c in SP_LOAD]
    pool_list = [c for c in range(n) if c not in SP_LOAD]
    act_sq = [c for c in range(n) if c not in DVE_SQ]
    pool_fold = [c for c in range(n) if c in POOL_FOLD]
    return n, off, sp_list, pool_list, act_sq, pool_fold


def _frames_view(bs_ap):
    """[128, NFRM, 10] view of the block-sum tile: frame f = blocks 3f..3f+9."""
    base = bs_ap[:, 0:1]
    return type(base)(
        tensor=base.tensor,
        offset=base.offset,
        ap=[list(base.ap[0]), [3, NFRM], [1, FRAME // BLK]],
    )


def _build_program() -> bass.Bass:
    nc = bass.Bass("TRN2", target_bir_lowering=False, debug=False)
    AF = mybir.ActivationFunctionType
    ALU = mybir.AluOpType
    AX = mybir.AxisListType

    n, off, sp_list, pool_list, act_sq, pool_fold = _sched()
    act_ord = {c: i for i, c in enumerate(act_sq)}
    pool_fold_ord = {c: i for i, c in enumerate(pool_fold)}
    sp_ord = {c: i for i, c in enumerate(sp_list)}
    pool_ord = {c: i for i, c in enumerate(pool_list)}

    xy = nc.dram_tensor("xy", [128, T], F32, kind="ExternalInput").ap()
    out = nc.dram_tensor("lufs", [128, 1], F32, kind="ExternalOutput").ap()

    xt32 = [
        nc.alloc_sbuf_tensor(f"xt32_{i}", [128, MAXW], F32).ap() for i in range(N_F32)
    ]
    xt16 = [
        nc.alloc_sbuf_tensor(f"xt16_{i}", [128, MAXW], BF16).ap() for i in range(N_B16)
    ]
    bs = nc.alloc_sbuf_tensor("bs", [128, NBLK_USED], F32).ap()
    zsum = nc.alloc_sbuf_tensor("zsum", [128, NFRM], F32).ap()
    z = nc.alloc_sbuf_tensor("z", [128, NFRM], F32).ap()
    ga = nc.alloc_sbuf_tensor("ga", [128, NFRM], F32).ap()
    ma = nc.alloc_sbuf_tensor("ma", [128, NFRM], F32).ap()
    gar = nc.alloc_sbuf_tensor("gar", [128, NFRM], F32).ap()
    junk = nc.alloc_sbuf_tensor("junk", [128, NFRM], F32).ap()
    sc = nc.alloc_sbuf_tensor("sc", [128, 12], F32).ap()
    eps_t = nc.alloc_sbuf_tensor("eps_t", [128, 1], F32).ap()

    numa = sc[:, 0:1]
    dena = sc[:, 1:2]
    rca = sc[:, 2:3]
    zavea = sc[:, 3:4]
    thr = sc[:, 4:5]
    denar = sc[:, 5:6]
    numar = sc[:, 6:7]
    rcar = sc[:, 7:8]
    zavear = sc[:, 8:9]
    lnz = sc[:, 9:10]
    lufs_t = sc[:, 10:11]

    def sq_tile(c):
        """bf16 view holding chunk c's squares (in-place in its input tile)."""
        if c in SP_LOAD:
            return xt32[sp_ord[c] % N_F32].bitcast(BF16)
        return xt16[pool_ord[c] % N_B16]

    def blkv(ap, w, b=BLK):
        return ap[:, 0:w].rearrange("p (n b) -> p n b", b=b)

    with (
        nc.Block() as block,
        nc.semaphore("s_f32_0") as s_f32_0,
        nc.semaphore("s_f32_1") as s_f32_1,
        nc.semaphore("s_b16_0") as s_b16_0,
        nc.semaphore("s_b16_1") as s_b16_1,
        nc.semaphore("s_b16_2") as s_b16_2,
        nc.semaphore("s_b16_3") as s_b16_3,
        nc.semaphore("s_b16_4") as s_b16_4,
        nc.semaphore("s_sqA") as s_sqA,
        nc.semaphore("s_sqD") as s_sqD,
        nc.semaphore("s_pf0") as s_pf0,
        nc.semaphore("s_pf1") as s_pf1,
        nc.semaphore("s_red") as s_red,
        nc.semaphore("s_zav") as s_zav,
        nc.semaphore("s_ln") as s_ln,
        nc.semaphore("s_lufs") as s_lufs,
        nc.semaphore("s_out") as s_out,
    ):
        s_f32 = [s_f32_0, s_f32_1]
        s_b16 = [s_b16_0, s_b16_1, s_b16_2, s_b16_3, s_b16_4]
        s_pf = [s_pf0, s_pf1]

        # ---- SP: f32 loads + final output DMA -----------------------------
        @block.sync
        def _(sy):
            for i, c in enumerate(sp_list):
                w = CHUNK_BLOCKS[c] * BLK
                slot = i % N_F32
                if i >= N_F32:
                    # slot free once its previous occupant's chunk was reduced
                    sy.wait_ge(s_red, sp_list[i - N_F32] + 1)
                sy.dma_start(
                    out=xt32[slot][:, 0:w], in_=xy[:, off[c] * BLK : off[c] * BLK + w]
                ).then_inc(s_f32[slot], 16)
            sy.wait_ge(s_lufs, 1)
            sy.dma_start(out=out, in_=lufs_t).then_inc(s_out, 16)
            sy.wait_ge(s_out, 16)

        # ---- Pool: bf16 cast loads + fold-1 accum DMAs --------------------
        @block.gpsimd
        def _(g):
            emitted = set()

            def maybe_fold(c):
                if c in POOL_FOLD and c not in emitted:
                    emitted.add(c)
                    k = pool_fold_ord[c]
                    w = CHUNK_BLOCKS[c] * BLK
                    t = sq_tile(c)
                    if k >= 2:
                        # >=1 fold DMA in flight max per sem: gate issue on the
                        # consumption (chunk reduce) of fold k-2
                        g.wait_ge(s_red, pool_fold[k - 2] + 1)
                    g.wait_ge(s_sqA, act_ord[c] + 1)
                    g.dma_start(
                        out=blkv(t, w)[:, :, 0 : BLK // 2],
                        in_=blkv(t, w)[:, :, BLK // 2 : BLK],
                        accum_op=mybir.AluOpType.add,
                    ).then_inc(s_pf[k % 2], 16)

            for i, c in enumerate(pool_list):
                w = CHUNK_BLOCKS[c] * BLK
                slot = i % N_B16
                if i >= N_B16:
                    g.wait_ge(s_red, pool_list[i - N_B16] + 1)
                g.dma_start(
                    out=xt16[slot][:, 0:w], in_=xy[:, off[c] * BLK : off[c] * BLK + w]
                ).then_inc(s_b16[slot], 16)
                # fold DMAs trail the load stream so their waits are satisfied
                for f in pool_fold:
                    if f <= c - 3:
                        maybe_fold(f)
            for f in pool_fold:
                maybe_fold(f)

        # ---- ACT: in-place squares ----------------------------------------
        @block.scalar
        def _(s):
            for i, c in enumerate(act_sq):
                w = CHUNK_BLOCKS[c] * BLK
                if c in SP_LOAD:
                    si = sp_ord[c]
                    s.wait_ge(s_f32[si % N_F32], (si // N_F32 + 1) * 16)
                    src = xt32[si % N_F32][:, 0:w]
                else:
                    pi = pool_ord[c]
                    s.wait_ge(s_b16[pi % N_B16], (pi // N_B16 + 1) * 16)
                    src = xt16[pi % N_B16][:, 0:w]
                s.activation(sq_tile(c)[:, 0:w], src, AF.Square)
                s.drain().then_inc(s_sqA, 1)
            # epilogue: ln(z_ave_ar + EPS)
            s.wait_ge(s_zav, 1)
            s.activation(lnz, zavear, AF.Ln, bias=eps_t)
            s.drain().then_inc(s_ln, 1)

        # ---- DVE: bf16 squares, fold tree, reduces, gating epilogue -------
        @block.vector
        def _(v):
            v.memset(eps_t, EPS)
            for c in range(n):
                nb = CHUNK_BLOCKS[c]
                w = nb * BLK
                t = sq_tile(c)
                if c in DVE_SQ:
                    pi = pool_ord[c]
                    v.wait_ge(s_b16[pi % N_B16], (pi // N_B16 + 1) * 16)
                    v.tensor_tensor(t[:, 0:w], t[:, 0:w], t[:, 0:w], op=ALU.mult)
                    v.drain().then_inc(s_sqD, 1)
                    wcur = BLK
                elif c in POOL_FOLD:
                    k = pool_fold_ord[c]
                    v.wait_ge(s_pf[k % 2], (k // 2 + 1) * 16)
                    wcur = BLK // 2
                else:
                    v.wait_ge(s_sqA, act_ord[c] + 1)
                    wcur = BLK
                # in-place fold tree: lo half += hi half within each block
                while wcur > 60:
                    h = wcur // 2
                    bv = blkv(t, w)
                    v.tensor_tensor(
                        bv[:, :, 0:h], bv[:, :, 0:h], bv[:, :, h:wcur], op=ALU.add
                    )
                    v.drain()
                    wcur = h
                v.reduce_sum(
                    bs[:, off[c] : off[c] + nb], blkv(t, w)[:, :, 0:60], axis=AX.X
                )
                v.drain().then_inc(s_red, 1)

            # ---- gating epilogue (all [128, NFRM] or [128, 1] ops) --------
            v.reduce_sum(zsum[:, :], _frames_view(bs), axis=AX.X)
            v.drain()
            v.tensor_scalar_mul(z[:, :], zsum[:, :], INV_FRAME)
            v.drain()
            v.scalar_tensor_tensor(
                out=ga[:, :], in0=z[:, :], scalar=TA, in1=z[:, :],
                op0=ALU.is_gt, op1=ALU.mult,
            )
            v.tensor_scalar(ma[:, :], z[:, :], TA, None, op0=ALU.is_gt)
            v.drain()
            v.reduce_sum(numa, ga[:, :], axis=AX.X)
            v.reduce_sum(dena, ma[:, :], axis=AX.X)
            v.drain()
            v.tensor_scalar_add(dena, dena, EPS)
            v.drain()
            v.reciprocal(rca, dena)
            v.drain()
            v.tensor_tensor(zavea, numa, rca, op=ALU.mult)
            v.drain()
            v.tensor_scalar(thr, zavea, 0.1, TR_OFF, op0=ALU.mult, op1=ALU.add)
            v.drain()
            v.scalar_tensor_tensor(
                out=gar[:, :], in0=z[:, :], scalar=thr, in1=ma[:, :],
                op0=ALU.is_gt, op1=ALU.mult,
            )
            v.drain()
            # z*gar = (z > thr) * ga, so reuse ga instead of a fresh multiply
            v.scalar_tensor_tensor(
                out=junk[:, :], in0=z[:, :], scalar=thr, in1=ga[:, :],
                op0=ALU.is_gt, op1=ALU.mult,
            )
            v.reduce_sum(denar, gar[:, :], axis=AX.X)
            v.drain()
            v.reduce_sum(numar, junk[:, :], axis=AX.X)
            v.drain()
            v.tensor_scalar_add(denar, denar, EPS)
            v.drain()
            v.reciprocal(rcar, denar)
            v.drain()
            v.tensor_tensor(zavear, numar, rcar, op=ALU.mult)
            v.drain().then_inc(s_zav, 1)
            v.wait_ge(s_ln, 1)
            v.tensor_scalar(
                lufs_t, lnz, LN10_INV10, -0.691, op0=ALU.mult, op1=ALU.add
            )
            v.drain().then_inc(s_lufs, 1)

    return nc


def make_in_maps(x_env: np.ndarray, y_env: np.ndarray) -> list[dict[str, np.ndarray]]:
    x = np.asarray(x_env, dtype=np.float32).reshape(ROWS, T)
    y = np.asarray(y_env, dtype=np.float32).reshape(ROWS, T)
    in_maps = []
    for i in range(N_CORES):
        shard = np.concatenate(
            [x[i * RPC : (i + 1) * RPC], y[i * RPC : (i + 1) * RPC]], axis=0
        )
        in_maps.append({"xy": np.ascontiguousarray(shard)})
    return in_maps


def finish(per_core_lufs: list[np.ndarray]) -> np.ndarray:
    total = 0.0
    for lf in per_core_lufs:
        lf = np.asarray(lf).reshape(128).astype(np.float64)
        total += np.maximum(lf[RPC:] - lf[:RPC], 0.0).sum()
    return np.array(ALPHA * total, dtype=np.float32)


def kernel(x_env: np.ndarray, y_env: np.ndarray) -> np.ndarray:
    nc = _build_program()
    in_maps = make_in_maps(x_env, y_env)
    res = run_bass_kernel_spmd(nc, in_maps, core_ids=list(range(N_CORES)))
    return finish([res.results[i]["lufs"] for i in range(N_CORES)])


# revision 15
# speedup vs baseline: 1.9385x; 1.1379x over previous
"""DHASPI level-loss kernel for 8 Trainium2 NeuronCores.

Data-parallel over the fused B*C row axis: each core processes 64 rows of
x_env (SBUF partitions 0-63) and 64 rows of y_env (partitions 64-127). Per
row the kernel computes the gated LUFS loudness; the final relu-diff scalar
sum over the 512 rows is done on the host from the 8 tiny [128, 1] outputs.

Math notes:
- Frame energies (9600-sample windows, shift 2880) are built from 960-sample
  block sums: gcd(9600, 2880) = 960, frame f = blocks 3f..3f+9. Only blocks
  0..198 are covered by any frame, so the last 960 samples of every row are
  never loaded (191040 of 192000 samples).
- All dB-domain gating comparisons are done in the energy domain via the
  monotone map el = -0.691 + 10*log10(z + eps), so the only transcendental
  on device is one Ln per row at the end.
- bf16 squares/folds put ~2e-4 relative noise on block sums (~0.001 dB on
  LUFS); the gating margins on this problem are >9 dB, far from any flip.

Engine-balanced dataflow (all four queues ~equal):
- Loads split between the SP queue (f32) and the GpSimd/Pool queue (SWDGE
  f32->bf16 cast during DMA, half the SBUF write traffic).
- Squares split between ACT (activation Square) and DVE (tensor_tensor mult
  in bf16, 2x perf mode). Squares are IN-PLACE: the input tile is squared
  into itself (for f32 inputs, into the tile's low half via a bf16 bitcast
  view - the bf16 write offset 2i trails the f32 read offset 4i, so the
  stream never clobbers unread data).
- The 960->60 per-block reduction is an in-place binary fold tree (bf16
  tensor_tensor adds at 2x; lo += hi within each block); fold-1 of some
  chunks is offloaded to Pool SBUF->SBUF DMAs with accum_op=add. A final
  1x tensor_reduce collapses 60 -> 1 per block into f32 block sums.

Raw Bass (explicit semaphores). Same-queue DMA completions are only safe to
track on one cumulative semaphore when never >1 in flight, so the Pool fold
DMAs rotate over two sems with issue gated on consumption of fold k-2.
"""

import math

import numpy as np

import concourse.bass as bass
from concourse import mybir
from concourse.bass_utils import run_bass_kernel_spmd

# Problem constants (hardcoded from the spec; kernel.py must be self-contained)
B, C, T = 16, 32, 192000
N_CORES = 8
ROWS = B * C  # 512
RPC = ROWS // N_CORES  # 64 rows per core per tensor

FRAME = 9600
SHIFT = 2880
BLK = 960
NBLK_USED = 199  # blocks 0..198 feed frames; block 199 is dead
NFRM = (T - FRAME) // SHIFT + 1  # 64

EPS = 1e-8
ALPHA = 1e-4
GAMMA_A = -70.0
TA = float(10.0 ** ((GAMMA_A + 0.691) / 10.0) - EPS)  # z-domain abs threshold
TR_OFF = float(0.1 * EPS - EPS)
LN10_INV10 = float(10.0 / math.log(10.0))
INV_FRAME = float(1.0 / FRAME)
# The gating epilogue runs directly on frame *sums* (zsum = z * FRAME):
# thresholds and the final log are rescaled by FRAME so no divide is needed.
TA_Z = float(TA * FRAME)
TR_OFF_Z = float(TR_OFF * FRAME)
EPS_Z = float(EPS * FRAME)
FINAL_C = float(-0.691 - LN10_INV10 * math.log(FRAME))

F32 = mybir.dt.float32
BF16 = mybir.dt.bfloat16

# ---- chunk schedule -------------------------------------------------------
# sizes in blocks; 28 chunks totalling 199 blocks. Small chunks at the start
# (fast pipeline fill from cheap Pool cast loads) and at the end (short
# post-last-DMA critical path: tiny DVE square + folds, then the epilogue).
CHUNK_BLOCKS = [5, 3, 1] + [8] * 23 + [4, 2]
# which chunks the SP queue loads as f32 (the rest are Pool bf16-cast loads)
SP_LOAD = {5, 7, 9, 11, 13, 15, 17, 19, 21, 23}
# which chunks DVE squares (must be Pool-cast loads); the rest ACT squares
DVE_SQ = {1, 2, 6, 10, 14, 18, 22, 26, 27}
# which chunks get fold-1 from a Pool SBUF->SBUF accum DMA (ACT-squared,
# kept away from the first and last chunks)
POOL_FOLD = {7, 8, 9, 11, 12, 13, 15, 16, 17, 19, 20, 21}

N_F32 = 3  # f32 input tile slots (SP loads)
N_B16 = 5  # bf16 input tile slots (Pool cast loads)
MAXW = 8 * BLK  # widest chunk


def _sched():
    n = len(CHUNK_BLOCKS)
    off = [sum(CHUNK_BLOCKS[:i]) for i in range(n)]
    sp_list = [c for c in range(n) if c in SP_LOAD]
    pool_list = [c for c in range(n) if c not in SP_LOAD]
    act_sq = [c for c in range(n) if c not in DVE_SQ]
    pool_fold = [c for c in range(n) if c in POOL_FOLD]
    return n, off, sp_list, pool_list, act_sq, pool_fold


def _frames_view(bs_ap):
    """[128, NFRM, 10] view of the block-sum tile: frame f = blocks 3f..3f+9."""
    base = bs_ap[:, 0:1]
    return type(base)(
        tensor=base.tensor,
        offset=base.offset,
        ap=[list(base.ap[0]), [3, NFRM], [1, FRAME // BLK]],
    )


def _build_program() -> bass.Bass:
    nc = bass.Bass("TRN2", target_bir_lowering=False, debug=False)
    AF = mybir.ActivationFunctionType
    ALU = mybir.AluOpType
    AX = mybir.AxisListType

    n, off, sp_list, pool_list, act_sq, pool_fold = _sched()
    act_ord = {c: i for i, c in enumerate(act_sq)}
    pool_fold_ord = {c: i for i, c in enumerate(pool_fold)}
    sp_ord = {c: i for i, c in enumerate(sp_list)}
    pool_ord = {c: i for i, c in enumerate(pool_list)}

    xy = nc.dram_tensor("xy", [128, T], F32, kind="ExternalInput").ap()
    out = nc.dram_tensor("lufs", [128, 1], F32, kind="ExternalOutput").ap()

    xt32 = [
        nc.alloc_sbuf_tensor(f"xt32_{i}", [128, MAXW], F32).ap() for i in range(N_F32)
    ]
    xt16 = [
        nc.alloc_sbuf_tensor(f"xt16_{i}", [128, MAXW], BF16).ap() for i in range(N_B16)
    ]
    bs = nc.alloc_sbuf_tensor("bs", [128, NBLK_USED], F32).ap()
    zsum = nc.alloc_sbuf_tensor("zsum", [128, NFRM], F32).ap()
    ga = nc.alloc_sbuf_tensor("ga", [128, NFRM], F32).ap()
    ma = nc.alloc_sbuf_tensor("ma", [128, NFRM], F32).ap()
    gar = nc.alloc_sbuf_tensor("gar", [128, NFRM], F32).ap()
    junk = nc.alloc_sbuf_tensor("junk", [128, NFRM], F32).ap()
    sc = nc.alloc_sbuf_tensor("sc", [128, 12], F32).ap()
    eps_t = nc.alloc_sbuf_tensor("eps_t", [128, 1], F32).ap()

    numa = sc[:, 0:1]
    dena = sc[:, 1:2]
    rca = sc[:, 2:3]
    zavea = sc[:, 3:4]
    thr = sc[:, 4:5]
    denar = sc[:, 5:6]
    numar = sc[:, 6:7]
    rcar = sc[:, 7:8]
    zavear = sc[:, 8:9]
    lnz = sc[:, 9:10]
    lufs_t = sc[:, 10:11]

    def sq_tile(c):
        """bf16 view holding chunk c's squares (in-place in its input tile)."""
        if c in SP_LOAD:
            return xt32[sp_ord[c] % N_F32].bitcast(BF16)
        return xt16[pool_ord[c] % N_B16]

    def blkv(ap, w, b=BLK):
        return ap[:, 0:w].rearrange("p (n b) -> p n b", b=b)

    with (
        nc.Block() as block,
        nc.semaphore("s_f32_0") as s_f32_0,
        nc.semaphore("s_f32_1") as s_f32_1,
        nc.semaphore("s_f32_2") as s_f32_2,
        nc.semaphore("s_b16_0") as s_b16_0,
        nc.semaphore("s_b16_1") as s_b16_1,
        nc.semaphore("s_b16_2") as s_b16_2,
        nc.semaphore("s_b16_3") as s_b16_3,
        nc.semaphore("s_b16_4") as s_b16_4,
        nc.semaphore("s_sqA") as s_sqA,
        nc.semaphore("s_sqD") as s_sqD,
        nc.semaphore("s_pf0") as s_pf0,
        nc.semaphore("s_pf1") as s_pf1,
        nc.semaphore("s_red") as s_red,
        nc.semaphore("s_zav") as s_zav,
        nc.semaphore("s_ln") as s_ln,
        nc.semaphore("s_lufs") as s_lufs,
        nc.semaphore("s_out") as s_out,
    ):
        s_f32 = [s_f32_0, s_f32_1, s_f32_2]
        s_b16 = [s_b16_0, s_b16_1, s_b16_2, s_b16_3, s_b16_4]
        s_pf = [s_pf0, s_pf1]

        # ---- SP: f32 loads + final output DMA -----------------------------
        @block.sync
        def _(sy):
            for i, c in enumerate(sp_list):
                w = CHUNK_BLOCKS[c] * BLK
                slot = i % N_F32
                if i >= N_F32:
                    # slot free once its previous occupant's chunk was reduced
                    sy.wait_ge(s_red, sp_list[i - N_F32] + 1)
                sy.dma_start(
                    out=xt32[slot][:, 0:w], in_=xy[:, off[c] * BLK : off[c] * BLK + w]
                ).then_inc(s_f32[slot], 16)
            sy.wait_ge(s_lufs, 1)
            sy.dma_start(out=out, in_=lufs_t).then_inc(s_out, 16)
            sy.wait_ge(s_out, 16)

        # ---- Pool: bf16 cast loads + fold-1 accum DMAs --------------------
        @block.gpsimd
        def _(g):
            emitted = set()

            def maybe_fold(c):
                if c in POOL_FOLD and c not in emitted:
                    emitted.add(c)
                    k = pool_fold_ord[c]
                    w = CHUNK_BLOCKS[c] * BLK
                    t = sq_tile(c)
                    if k >= 2:
                        # >=1 fold DMA in flight max per sem: gate issue on the
                        # consumption (chunk reduce) of fold k-2
                        g.wait_ge(s_red, pool_fold[k - 2] + 1)
                    g.wait_ge(s_sqA, act_ord[c] + 1)
                    g.dma_start(
                        out=blkv(t, w)[:, :, 0 : BLK // 2],
                        in_=blkv(t, w)[:, :, BLK // 2 : BLK],
                        accum_op=mybir.AluOpType.add,
                    ).then_inc(s_pf[k % 2], 16)

            for i, c in enumerate(pool_list):
                w = CHUNK_BLOCKS[c] * BLK
                slot = i % N_B16
                if i >= N_B16:
                    g.wait_ge(s_red, pool_list[i - N_B16] + 1)
                g.dma_start(
                    out=xt16[slot][:, 0:w], in_=xy[:, off[c] * BLK : off[c] * BLK + w]
                ).then_inc(s_b16[slot], 16)
                # fold DMAs trail the load stream so their waits are satisfied
                for f in pool_fold:
                    if f <= c - 3:
                        maybe_fold(f)
            for f in pool_fold:
                maybe_fold(f)

        # ---- ACT: in-place squares ----------------------------------------
        @block.scalar
        def _(s):
            for i, c in enumerate(act_sq):
                w = CHUNK_BLOCKS[c] * BLK
                if c in SP_LOAD:
                    si = sp_ord[c]
                    s.wait_ge(s_f32[si % N_F32], (si // N_F32 + 1) * 16)
                    src = xt32[si % N_F32][:, 0:w]
                else:
                    pi = pool_ord[c]
                    s.wait_ge(s_b16[pi % N_B16], (pi // N_B16 + 1) * 16)
                    src = xt16[pi % N_B16][:, 0:w]
                s.activation(sq_tile(c)[:, 0:w], src, AF.Square)
                s.drain().then_inc(s_sqA, 1)
            # epilogue: ln(z_ave_ar + EPS)
            s.wait_ge(s_zav, 1)
            s.activation(lnz, zavear, AF.Ln, bias=eps_t)
            s.drain().then_inc(s_ln, 1)

        # ---- DVE: bf16 squares, fold tree, reduces, gating epilogue -------
        @block.vector
        def _(v):
            v.memset(eps_t, EPS_Z)
            for c in range(n):
                nb = CHUNK_BLOCKS[c]
                w = nb * BLK
                t = sq_tile(c)
                if c in DVE_SQ:
                    pi = pool_ord[c]
                    v.wait_ge(s_b16[pi % N_B16], (pi // N_B16 + 1) * 16)
                    v.tensor_tensor(t[:, 0:w], t[:, 0:w], t[:, 0:w], op=ALU.mult)
                    v.drain().then_inc(s_sqD, 1)
                    wcur = BLK
                elif c in POOL_FOLD:
                    k = pool_fold_ord[c]
                    v.wait_ge(s_pf[k % 2], (k // 2 + 1) * 16)
                    wcur = BLK // 2
                else:
                    v.wait_ge(s_sqA, act_ord[c] + 1)
                    wcur = BLK
                # in-place fold tree: lo half += hi half within each block
                while wcur > 60:
                    h = wcur // 2
                    bv = blkv(t, w)
                    v.tensor_tensor(
                        bv[:, :, 0:h], bv[:, :, 0:h], bv[:, :, h:wcur], op=ALU.add
                    )
                    v.drain()
                    wcur = h
                v.reduce_sum(
                    bs[:, off[c] : off[c] + nb], blkv(t, w)[:, :, 0:60], axis=AX.X
                )
                v.drain().then_inc(s_red, 1)

            # ---- gating epilogue, all in the zsum (= z * FRAME) domain ----
            v.reduce_sum(zsum[:, :], _frames_view(bs), axis=AX.X)
            v.drain()
            v.scalar_tensor_tensor(
                out=ga[:, :], in0=zsum[:, :], scalar=TA_Z, in1=zsum[:, :],
                op0=ALU.is_gt, op1=ALU.mult,
            )
            v.tensor_scalar(ma[:, :], zsum[:, :], TA_Z, None, op0=ALU.is_gt)
            v.drain()
            v.reduce_sum(numa, ga[:, :], axis=AX.X)
            v.reduce_sum(dena, ma[:, :], axis=AX.X)
            v.drain()
            v.tensor_scalar_add(dena, dena, EPS)
            v.drain()
            v.reciprocal(rca, dena)
            v.drain()
            v.tensor_tensor(zavea, numa, rca, op=ALU.mult)
            v.drain()
            v.tensor_scalar(thr, zavea, 0.1, TR_OFF_Z, op0=ALU.mult, op1=ALU.add)
            v.drain()
            v.scalar_tensor_tensor(
                out=gar[:, :], in0=zsum[:, :], scalar=thr, in1=ma[:, :],
                op0=ALU.is_gt, op1=ALU.mult,
            )
            v.drain()
            # zsum*gar = (zsum > thr) * ga, so reuse ga, no fresh multiply
            v.scalar_tensor_tensor(
                out=junk[:, :], in0=zsum[:, :], scalar=thr, in1=ga[:, :],
                op0=ALU.is_gt, op1=ALU.mult,
            )
            v.reduce_sum(denar, gar[:, :], axis=AX.X)
            v.drain()
            v.reduce_sum(numar, junk[:, :], axis=AX.X)
            v.drain()
            v.tensor_scalar_add(denar, denar, EPS)
            v.drain()
            v.reciprocal(rcar, denar)
            v.drain()
            v.tensor_tensor(zavear, numar, rcar, op=ALU.mult)
            v.drain().then_inc(s_zav, 1)
            v.wait_ge(s_ln, 1)
            v.tensor_scalar(
                lufs_t, lnz, LN10_INV10, FINAL_C, op0=ALU.mult, op1=ALU.add
            )
            v.drain().then_inc(s_lufs, 1)

    return nc


def make_in_maps(x_env: np.ndarray, y_env: np.ndarray) -> list[dict[str, np.ndarray]]:
    x = np.asarray(x_env, dtype=np.float32).reshape(ROWS, T)
    y = np.asarray(y_env, dtype=np.float32).reshape(ROWS, T)
    in_maps = []
    for i in range(N_CORES):
        shard = np.concatenate(
            [x[i * RPC : (i + 1) * RPC], y[i * RPC : (i + 1) * RPC]], axis=0
        )
        in_maps.append({"xy": np.ascontiguousarray(shard)})
    return in_maps


def finish(per_core_lufs: list[np.ndarray]) -> np.ndarray:
    total = 0.0
    for lf in per_core_lufs:
        lf = np.asarray(lf).reshape(128).astype(np.float64)
        total += np.maximum(lf[RPC:] - lf[:RPC], 0.0).sum()
    return np.array(ALPHA * total, dtype=np.float32)


def kernel(x_env: np.ndarray, y_env: np.ndarray) -> np.ndarray:
    nc = _build_program()
    in_maps = make_in_maps(x_env, y_env)
    res = run_bass_kernel_spmd(nc, in_maps, core_ids=list(range(N_CORES)))
    return finish([res.results[i]["lufs"] for i in range(N_CORES)])


# revision 17
# speedup vs baseline: 2.1135x; 1.0903x over previous
"""DHASPI level-loss kernel for 8 Trainium2 NeuronCores.

Data-parallel over the fused B*C row axis: each core processes 64 rows of
x_env (SBUF partitions 0-63) and 64 rows of y_env (partitions 64-127). Per
row the kernel computes the gated LUFS loudness; the final relu-diff scalar
sum over the 512 rows is done on the host from the 8 tiny [128, 1] outputs.

Math notes:
- Frame energies (9600-sample windows, shift 2880) are built from 960-sample
  block sums: gcd(9600, 2880) = 960, frame f = blocks 3f..3f+9. Only blocks
  0..198 are covered by any frame, so the last 960 samples of every row are
  never loaded (191040 of 192000 samples).
- All dB-domain gating comparisons are done in the energy (frame-sum) domain
  via the monotone map el = -0.691 + 10*log10(z + eps), so the only
  transcendental on device is one Ln per row at the end.
- bf16 squares put ~2e-4 relative noise on block sums (~0.001 dB on LUFS);
  the gating margins on this problem are >9 dB, far from any flip.

Engine-balanced dataflow (all four queues ~equal):
- Loads split between the SP queue (f32) and the GpSimd/Pool queue (SWDGE
  f32->bf16 cast during DMA, half the SBUF write traffic).
- Squares split between ACT (activation Square) and DVE (tensor_tensor mult
  in bf16, 2x perf mode). Squares are IN-PLACE: the input tile is squared
  into itself (for f32 inputs, into the tile's low half via a bf16 bitcast
  view - the bf16 write offset 2i trails the f32 read offset 4i, so the
  stream never clobbers unread data).
- Block sums: one bf16 tensor_scalar per 960-block with accum_out into the
  f32 block-sum tile. bf16 single-source tensor_scalar streams at 4x
  (0.26 ns/elem) and the accumulator is fp32, so this replaces a whole
  fold-tree at a third of the cost and with no extra rounding.

Raw Bass (explicit semaphores); cross-engine RAW uses drain() before
then_inc, per-slot DMA semaphores, in-order queues carry the rest.
"""

import math

import numpy as np

import concourse.bass as bass
from concourse import mybir
from concourse.bass_utils import run_bass_kernel_spmd

# Problem constants (hardcoded from the spec; kernel.py must be self-contained)
B, C, T = 16, 32, 192000
N_CORES = 8
ROWS = B * C  # 512
RPC = ROWS // N_CORES  # 64 rows per core per tensor

FRAME = 9600
SHIFT = 2880
BLK = 960
NBLK_USED = 199  # blocks 0..198 feed frames; block 199 is dead
NFRM = (T - FRAME) // SHIFT + 1  # 64

EPS = 1e-8
ALPHA = 1e-4
GAMMA_A = -70.0
TA = float(10.0 ** ((GAMMA_A + 0.691) / 10.0) - EPS)  # z-domain abs threshold
TR_OFF = float(0.1 * EPS - EPS)
LN10_INV10 = float(10.0 / math.log(10.0))
INV_FRAME = float(1.0 / FRAME)
# The gating epilogue runs directly on frame *sums* (zsum = z * FRAME):
# thresholds and the final log are rescaled by FRAME so no divide is needed.
TA_Z = float(TA * FRAME)
TR_OFF_Z = float(TR_OFF * FRAME)
EPS_Z = float(EPS * FRAME)
FINAL_C = float(-0.691 - LN10_INV10 * math.log(FRAME))

F32 = mybir.dt.float32
BF16 = mybir.dt.bfloat16

# ---- chunk schedule -------------------------------------------------------
# sizes in blocks; 28 chunks totalling 199 blocks. Small chunks at the start
# (fast pipeline fill from cheap Pool cast loads) and at the end (short
# post-last-DMA critical path: tiny DVE square + block sums, then epilogue).
CHUNK_BLOCKS = [5, 3, 1] + [8] * 23 + [4, 2]
# which chunks the SP queue loads as f32 (the rest are Pool bf16-cast loads)
SP_LOAD = {5, 7, 9, 11, 13, 15, 17, 19, 21}
# which chunks DVE squares (must be Pool-cast loads); the rest ACT squares
DVE_SQ = {1, 2, 4, 6, 8, 12, 16, 20, 24, 26, 27}
# DVE processes chunks in this order (early minis first so DVE starts on its
# own squares while ACT's first square is still in flight)
DVE_ORDER = [1, 2, 0] + list(range(3, 28))

N_F32 = 3  # f32 input tile slots (SP loads)
N_B16 = 5  # bf16 input tile slots (Pool cast loads)
MAXW = 8 * BLK  # widest chunk


def _sched():
    n = len(CHUNK_BLOCKS)
    off = [sum(CHUNK_BLOCKS[:i]) for i in range(n)]
    sp_list = [c for c in range(n) if c in SP_LOAD]
    pool_list = [c for c in range(n) if c not in SP_LOAD]
    act_sq = [c for c in range(n) if c not in DVE_SQ]
    return n, off, sp_list, pool_list, act_sq


def _frames_view(bs_ap):
    """[128, NFRM, 10] view of the block-sum tile: frame f = blocks 3f..3f+9."""
    base = bs_ap[:, 0:1]
    return type(base)(
        tensor=base.tensor,
        offset=base.offset,
        ap=[list(base.ap[0]), [3, NFRM], [1, FRAME // BLK]],
    )


def _build_program() -> bass.Bass:
    nc = bass.Bass("TRN2", target_bir_lowering=False, debug=False)
    AF = mybir.ActivationFunctionType
    ALU = mybir.AluOpType
    AX = mybir.AxisListType

    n, off, sp_list, pool_list, act_sq = _sched()
    act_ord = {c: i for i, c in enumerate(act_sq)}
    sp_ord = {c: i for i, c in enumerate(sp_list)}
    pool_ord = {c: i for i, c in enumerate(pool_list)}
    assert sorted(DVE_ORDER) == list(range(n))
    red_pos = {c: i for i, c in enumerate(DVE_ORDER)}  # s_red ordinal per chunk

    xy = nc.dram_tensor("xy", [128, T], F32, kind="ExternalInput").ap()
    out = nc.dram_tensor("lufs", [128, 1], F32, kind="ExternalOutput").ap()

    xt32 = [
        nc.alloc_sbuf_tensor(f"xt32_{i}", [128, MAXW], F32).ap() for i in range(N_F32)
    ]
    xt16 = [
        nc.alloc_sbuf_tensor(f"xt16_{i}", [128, MAXW], BF16).ap() for i in range(N_B16)
    ]
    bs = nc.alloc_sbuf_tensor("bs", [128, NBLK_USED], F32).ap()
    junk16 = nc.alloc_sbuf_tensor("junk16", [128, BLK], BF16).ap()
    zsum = nc.alloc_sbuf_tensor("zsum", [128, NFRM], F32).ap()
    ga = nc.alloc_sbuf_tensor("ga", [128, NFRM], F32).ap()
    ma = nc.alloc_sbuf_tensor("ma", [128, NFRM], F32).ap()
    gar = nc.alloc_sbuf_tensor("gar", [128, NFRM], F32).ap()
    junk = nc.alloc_sbuf_tensor("junk", [128, NFRM], F32).ap()
    sc = nc.alloc_sbuf_tensor("sc", [128, 12], F32).ap()
    eps_t = nc.alloc_sbuf_tensor("eps_t", [128, 1], F32).ap()

    numa = sc[:, 0:1]
    dena = sc[:, 1:2]
    rca = sc[:, 2:3]
    zavea = sc[:, 3:4]
    thr = sc[:, 4:5]
    denar = sc[:, 5:6]
    numar = sc[:, 6:7]
    rcar = sc[:, 7:8]
    zavear = sc[:, 8:9]
    lnz = sc[:, 9:10]
    lufs_t = sc[:, 10:11]

    def sq_tile(c):
        """bf16 view holding chunk c's squares (in-place in its input tile)."""
        if c in SP_LOAD:
            return xt32[sp_ord[c] % N_F32].bitcast(BF16)
        return xt16[pool_ord[c] % N_B16]

    with (
        nc.Block() as block,
        nc.semaphore("s_f32_0") as s_f32_0,
        nc.semaphore("s_f32_1") as s_f32_1,
        nc.semaphore("s_f32_2") as s_f32_2,
        nc.semaphore("s_b16_0") as s_b16_0,
        nc.semaphore("s_b16_1") as s_b16_1,
        nc.semaphore("s_b16_2") as s_b16_2,
        nc.semaphore("s_b16_3") as s_b16_3,
        nc.semaphore("s_b16_4") as s_b16_4,
        nc.semaphore("s_sqA") as s_sqA,
        nc.semaphore("s_red") as s_red,
        nc.semaphore("s_zav") as s_zav,
        nc.semaphore("s_lufs") as s_lufs,
        nc.semaphore("s_out") as s_out,
    ):
        s_f32 = [s_f32_0, s_f32_1, s_f32_2]
        s_b16 = [s_b16_0, s_b16_1, s_b16_2, s_b16_3, s_b16_4]

        # ---- SP: f32 loads + final output DMA -----------------------------
        @block.sync
        def _(sy):
            for i, c in enumerate(sp_list):
                w = CHUNK_BLOCKS[c] * BLK
                slot = i % N_F32
                if i >= N_F32:
                    # slot free once its previous occupant's chunk was summed
                    sy.wait_ge(s_red, red_pos[sp_list[i - N_F32]] + 1)
                sy.dma_start(
                    out=xt32[slot][:, 0:w], in_=xy[:, off[c] * BLK : off[c] * BLK + w]
                ).then_inc(s_f32[slot], 16)
            sy.wait_ge(s_lufs, 1)
            sy.dma_start(out=out, in_=lufs_t).then_inc(s_out, 16)
            sy.wait_ge(s_out, 16)

        # ---- Pool: bf16 cast loads ----------------------------------------
        @block.gpsimd
        def _(g):
            for i, c in enumerate(pool_list):
                w = CHUNK_BLOCKS[c] * BLK
                slot = i % N_B16
                if i >= N_B16:
                    g.wait_ge(s_red, red_pos[pool_list[i - N_B16]] + 1)
                g.dma_start(
                    out=xt16[slot][:, 0:w], in_=xy[:, off[c] * BLK : off[c] * BLK + w]
                ).then_inc(s_b16[slot], 16)

        # ---- ACT: in-place squares + final Ln/affine ----------------------
        @block.scalar
        def _(s):
            for i, c in enumerate(act_sq):
                w = CHUNK_BLOCKS[c] * BLK
                if c in SP_LOAD:
                    si = sp_ord[c]
                    s.wait_ge(s_f32[si % N_F32], (si // N_F32 + 1) * 16)
                    src = xt32[si % N_F32][:, 0:w]
                else:
                    pi = pool_ord[c]
                    s.wait_ge(s_b16[pi % N_B16], (pi // N_B16 + 1) * 16)
                    src = xt16[pi % N_B16][:, 0:w]
                s.activation(sq_tile(c)[:, 0:w], src, AF.Square)
                s.drain().then_inc(s_sqA, 1)
            # epilogue: lufs = (10/ln10)*ln(z_ave_ar + EPS) + FINAL_C
            s.wait_ge(s_zav, 1)
            s.activation(lnz, zavear, AF.Ln, bias=eps_t)
            s.drain()
            s.activation(lufs_t, lnz, AF.Copy, bias=FINAL_C, scale=LN10_INV10)
            s.drain().then_inc(s_lufs, 1)

        # ---- DVE: bf16 squares, per-block accum sums, gating epilogue -----
        @block.vector
        def _(v):
            v.memset(eps_t, EPS_Z)
            for c in DVE_ORDER:
                nb = CHUNK_BLOCKS[c]
                w = nb * BLK
                t = sq_tile(c)
                if c in DVE_SQ:
                    pi = pool_ord[c]
                    v.wait_ge(s_b16[pi % N_B16], (pi // N_B16 + 1) * 16)
                    v.tensor_tensor(t[:, 0:w], t[:, 0:w], t[:, 0:w], op=ALU.mult)
                    v.drain()
                else:
                    v.wait_ge(s_sqA, act_ord[c] + 1)
                # one 4x-rate bf16 tensor_scalar per block, fp32 accum -> bs
                for b in range(nb):
                    v.tensor_scalar(
                        junk16, t[:, b * BLK : (b + 1) * BLK], 1.0, 0.0,
                        op0=ALU.mult, op1=ALU.add,
                        accum_out=bs[:, off[c] + b : off[c] + b + 1],
                    )
                v.drain().then_inc(s_red, 1)

            # ---- gating epilogue, all in the zsum (= z * FRAME) domain ----
            v.reduce_sum(zsum[:, :], _frames_view(bs), axis=AX.X)
            v.drain()
            v.scalar_tensor_tensor(
                out=ga[:, :], in0=zsum[:, :], scalar=TA_Z, in1=zsum[:, :],
                op0=ALU.is_gt, op1=ALU.mult,
            )
            v.tensor_scalar(ma[:, :], zsum[:, :], TA_Z, None, op0=ALU.is_gt)
            v.drain()
            v.reduce_sum(numa, ga[:, :], axis=AX.X)
            v.reduce_sum(dena, ma[:, :], axis=AX.X)
            v.drain()
            v.tensor_scalar_add(dena, dena, EPS)
            v.drain()
            v.reciprocal(rca, dena)
            v.drain()
            v.tensor_tensor(zavea, numa, rca, op=ALU.mult)
            v.drain()
            v.tensor_scalar(thr, zavea, 0.1, TR_OFF_Z, op0=ALU.mult, op1=ALU.add)
            v.drain()
            v.scalar_tensor_tensor(
                out=gar[:, :], in0=zsum[:, :], scalar=thr, in1=ma[:, :],
                op0=ALU.is_gt, op1=ALU.mult,
            )
            v.drain()
            # zsum*gar = (zsum > thr) * ga, so reuse ga, no fresh multiply
            v.scalar_tensor_tensor(
                out=junk[:, :], in0=zsum[:, :], scalar=thr, in1=ga[:, :],
                op0=ALU.is_gt, op1=ALU.mult,
            )
            v.reduce_sum(denar, gar[:, :], axis=AX.X)
            v.drain()
            v.reduce_sum(numar, junk[:, :], axis=AX.X)
            v.drain()
            v.tensor_scalar_add(denar, denar, EPS)
            v.drain()
            v.reciprocal(rcar, denar)
            v.drain()
            v.tensor_tensor(zavear, numar, rcar, op=ALU.mult)
            v.drain().then_inc(s_zav, 1)

    return nc


def make_in_maps(x_env: np.ndarray, y_env: np.ndarray) -> list[dict[str, np.ndarray]]:
    x = np.asarray(x_env, dtype=np.float32).reshape(ROWS, T)
    y = np.asarray(y_env, dtype=np.float32).reshape(ROWS, T)
    in_maps = []
    for i in range(N_CORES):
        shard = np.concatenate(
            [x[i * RPC : (i + 1) * RPC], y[i * RPC : (i + 1) * RPC]], axis=0
        )
        in_maps.append({"xy": np.ascontiguousarray(shard)})
    return in_maps


def finish(per_core_lufs: list[np.ndarray]) -> np.ndarray:
    total = 0.0
    for lf in per_core_lufs:
        lf = np.asarray(lf).reshape(128).astype(np.float64)
        total += np.maximum(lf[RPC:] - lf[:RPC], 0.0).sum()
    return np.array(ALPHA * total, dtype=np.float32)


def kernel(x_env: np.ndarray, y_env: np.ndarray) -> np.ndarray:
    nc = _build_program()
    in_maps = make_in_maps(x_env, y_env)
    res = run_bass_kernel_spmd(nc, in_maps, core_ids=list(range(N_CORES)))
    return finish([res.results[i]["lufs"] for i in range(N_CORES)])
